# revision 13
# baseline (speedup 1.0000x reference)
"""Causal single-head attention (B=8, T=2048, C=1024, H=128) on 8 TRN2 NeuronCores.

Sharding: data-parallel over batch B — one batch element per core; weights
replicated. Inputs are cast to fp16 on the host (halves DMA, full-rate PE);
all matmuls accumulate in fp32 PSUM, softmax/normalization in fp32.

Per-core kernel:
  phase 1: x^T tiles via PE transposes; q^T,k^T = W.T @ x^T ([H,T] layout);
           v natural [T,H] via v^T + PE transposes.
  phase 2 (per 512-query block): s^T chunk = k_chunk @ q^T  -> exp (ACT,
           scale=C^-0.5; no max subtraction needed: |s/32| < ~2.5) ->
           causal mask on diagonal chunks (gpsimd affine_select) ->
           l += ones.T @ p^T and o^T += v_chunk.T @ p^T (PSUM accum) ->
           epilogue: transpose o^T -> o, scale rows by 1/l, DMA out.
"""
import numpy as np

import concourse.bass as bass
import concourse.mybir as mybir
import concourse.tile as tile
from concourse import bacc
from concourse.bass_utils import run_bass_kernel_spmd
from concourse.masks import make_identity

P = 128
T = 2048
C = 1024
H = 128
CO = C // P          # 8 contraction chunks
TB = 512             # T block for phase 1
NTB = T // TB        # 4
QB = 512             # query block for phase 2
NQB = T // QB        # 4
NKC = T // P         # 16 key chunks
F32 = mybir.dt.float32
F16 = mybir.dt.float16
SCALE = C ** -0.5    # 1/32, matches reference (scales by n_embed, not head_size)

N_CORES = 8


def _copy(nc, idx, out, in_):
    """Alternate psum->sbuf copies between DVE and ACT to halve copy pressure."""
    if idx % 2 == 0:
        nc.vector.tensor_copy(out, in_)
    else:
        nc.scalar.activation(out, in_, mybir.ActivationFunctionType.Copy)


def build_nc(s_bufs=3, misc_bufs=3, stage_bufs=2, ptile_bufs=6, xload_bufs=16):
    nc = bacc.Bacc("TRN2", target_bir_lowering=False, debug=False,
                   enable_asserts=False, num_devices=N_CORES)
    x = nc.dram_tensor("x", [T, C], F16, kind="ExternalInput")
    wq = nc.dram_tensor("Wq", [C, H], F16, kind="ExternalInput")
    wk = nc.dram_tensor("Wk", [C, H], F16, kind="ExternalInput")
    wv = nc.dram_tensor("Wv", [C, H], F16, kind="ExternalInput")
    out = nc.dram_tensor("out", [T, H], F32, kind="ExternalOutput")

    x4 = x.rearrange("(r p) (o c) -> p r o c", p=P, c=P)    # [128, 16, 8, 128]
    out3 = out.rearrange("(n p) h -> p n h", p=P)           # [128, 16, 128]

    with tile.TileContext(nc) as tc:
        with (
            tc.tile_pool(name="const", bufs=1) as const,
            tc.tile_pool(name="persist", bufs=1) as persist,
            tc.tile_pool(name="xload", bufs=xload_bufs) as xload,
            tc.tile_pool(name="stage", bufs=stage_bufs) as stage,
            tc.tile_pool(name="ptile", bufs=ptile_bufs) as ptile,
            tc.tile_pool(name="epi", bufs=2) as epi,
            tc.tile_pool(name="ps_acc", bufs=1, space="PSUM") as ps_acc,
            tc.tile_pool(name="ps_s", bufs=s_bufs, space="PSUM") as ps_s,
            tc.tile_pool(name="ps_misc", bufs=misc_bufs, space="PSUM") as ps_misc,
        ):
            # ---- constants ----
            ident = const.tile([P, P], F32)
            make_identity(nc, ident)
            identh = const.tile([P, P], F16)
            nc.vector.tensor_copy(identh[:], ident[:])
            ones_f = const.tile([P, 1], F32)
            nc.gpsimd.memset(ones_f[:], 1.0)
            ones_h = const.tile([P, 1], F16)
            nc.vector.tensor_copy(ones_h[:], ones_f[:])

            # ---- persistent activations ----
            q_T = persist.tile([P, T], F16)          # [H, T]
            k_T = persist.tile([P, T], F16)          # [H, T]
            v_nat = persist.tile([P, NKC, H], F16)   # [t%128, kc, H]

            # ---- x loads for tb0 first (startup latency), then weights ----
            x_blks = {}

            def load_xb(i):
                halves = []
                for half in range(2):
                    xh = xload.tile([P, 4, P], F16, name="xh")  # [t, o_half, c]
                    nc.sync.dma_start(xh[:], x4[:, i, half * 4:(half + 1) * 4, :])
                    halves.append(xh)
                x_blks[i] = halves

            for r in range(4):
                load_xb(r)

            w_tiles = []
            for nm, wd in (("wqt", wq), ("wkt", wk), ("wvt", wv)):
                wt = const.tile([P, CO, H], F16, name=nm)
                nc.sync.dma_start(wt[:], wd.rearrange("(o p) h -> p o h", p=P))
                w_tiles.append(wt)
            wq_t, wk_t, wv_t = w_tiles

            # ================= phase 1: projections =================
            cpy = 0
            for tb in range(NTB):
                xT = stage.tile([P, CO, TB], F16, name="xT")  # [c_in_chunk, o, t]
                for r in range(4):
                    if tb * 4 + r not in x_blks:
                        load_xb(tb * 4 + r)
                # per r-block: transpose its 8 chunks (2 psum tiles), copy out
                for r in range(4):
                    for half in range(2):
                        xh = x_blks[tb * 4 + r][half]
                        ps_x = ps_misc.tile([P, TB], F16, name="ps_x", tag="mps")
                        for cl in range(4):
                            nc.tensor.transpose(
                                ps_x[:, cl * P:(cl + 1) * P], xh[:, cl, :], identh[:])
                        _copy(nc, cpy, xT[:, half * 4:(half + 1) * 4, r * P:(r + 1) * P],
                              ps_x[:].rearrange("p (c t) -> p c t", t=P))
                        cpy += 1

                tsl = slice(tb * TB, (tb + 1) * TB)
                for wt, dest in ((wq_t, q_T), (wk_t, k_T)):
                    ps_p = ps_misc.tile([P, TB], F32, name="ps_p", tag="mps")
                    for c in range(CO):
                        nc.tensor.matmul(ps_p[:], wt[:, c, :], xT[:, c, :],
                                         start=(c == 0), stop=(c == CO - 1))
                    _copy(nc, cpy, dest[:, tsl], ps_p[:])
                    cpy += 1

                # v^T then transpose to natural
                ps_v = ps_misc.tile([P, TB], F32, name="ps_v", tag="mps")
                for c in range(CO):
                    nc.tensor.matmul(ps_v[:], wv_t[:, c, :], xT[:, c, :],
                                     start=(c == 0), stop=(c == CO - 1))
                vT_sb = stage.tile([P, TB], F16, name="vT_sb")
                _copy(nc, cpy, vT_sb[:], ps_v[:])
                cpy += 1
                ps_vn = ps_misc.tile([P, TB], F16, name="ps_vn", tag="mps")
                for j in range(4):
                    nc.tensor.transpose(
                        ps_vn[:, j * P:(j + 1) * P], vT_sb[:, j * P:(j + 1) * P], identh[:])
                _copy(nc, cpy, v_nat[:, tb * 4:(tb + 1) * 4, :],
                      ps_vn[:].rearrange("p (j h) -> p j h", h=H))
                cpy += 1

            # ================= phase 2: attention =================
            for b in range(NQB):
                nkc = 4 * (b + 1)
                o_ps = ps_acc.tile([P, QB], F32, name="o_ps")
                l_ps = ps_acc.tile([1, QB], F32, name="l_ps")
                for kc in range(nkc):
                    d = kc - 4 * b
                    off = max(d, 0) * P      # diagonal chunks: only queries >= key chunk start
                    w = QB - off
                    s_ps = ps_s.tile([P, QB], F32, name="s_ps")
                    nc.tensor.matmul(s_ps[:, :w], k_T[:, kc * P:(kc + 1) * P],
                                     q_T[:, b * QB + off:(b + 1) * QB],
                                     start=True, stop=True)
                    pT = ptile.tile([P, QB], F16, name="pT")
                    nc.scalar.activation(pT[:, off:], s_ps[:, :w],
                                         mybir.ActivationFunctionType.Exp, scale=SCALE)
                    if d >= 0:  # diagonal chunk: zero where key > query
                        nc.gpsimd.affine_select(
                            out=pT[:, off:], in_=pT[:, off:],
                            compare_op=mybir.AluOpType.is_ge,
                            fill=0.0, base=0,
                            pattern=[[1, w]], channel_multiplier=-1)
                    nc.tensor.matmul(l_ps[:, off:], ones_h[:], pT[:, off:],
                                     start=(kc == 0), stop=(kc == nkc - 1))
                    nc.tensor.matmul(o_ps[:, off:], v_nat[:, kc, :], pT[:, off:],
                                     start=(kc == 0), stop=(kc == nkc - 1))

                # epilogue: l -> linv [128, 4]; o^T -> o natural; scale; DMA out
                l_sb = epi.tile([1, QB], F32, name="l_sb")
                nc.vector.tensor_copy(l_sb[:], l_ps[:])
                ps_l = ps_misc.tile([P, 4], F32, name="ps_l", tag="mps")
                for j in range(4):
                    nc.tensor.transpose(ps_l[:, j:j + 1], l_sb[:, j * P:(j + 1) * P],
                                        ident[:1, :1])
                l_nat = epi.tile([P, 4], F32, name="l_nat")
                nc.vector.tensor_copy(l_nat[:], ps_l[:])
                linv = epi.tile([P, 4], F32, name="linv")
                nc.vector.reciprocal(linv[:], l_nat[:])

                oT_sb = epi.tile([P, QB], F32, name="oT_sb")
                o_nat = epi.tile([P, 4, H], F32, name="o_nat")
                for hf in range(2):
                    hsl = slice(hf * (QB // 2), (hf + 1) * (QB // 2))
                    nc.vector.tensor_copy(oT_sb[:, hsl], o_ps[:, hsl])
                    ps_on = ps_misc.tile([P, QB // 2], F32, name="ps_on", tag="mps")
                    for jj in range(2):
                        j = hf * 2 + jj
                        nc.tensor.transpose(
                            ps_on[:, jj * P:(jj + 1) * P], oT_sb[:, j * P:(j + 1) * P],
                            ident[:])
                    nc.vector.tensor_tensor(
                        o_nat[:, hf * 2:(hf + 1) * 2, :],
                        ps_on[:].rearrange("p (j h) -> p j h", h=H),
                        linv[:, hf * 2:(hf + 1) * 2, None].to_broadcast([P, 2, H]),
                        mybir.AluOpType.mult)
                    nc.sync.dma_start(out3[:, b * 4 + hf * 2:b * 4 + (hf + 1) * 2, :],
                                      o_nat[:, hf * 2:(hf + 1) * 2, :])

    nc.compile()
    return nc


_NC = None


def _get_nc():
    global _NC
    if _NC is None:
        _NC = build_nc()
    return _NC


def kernel(x, Wq, Wk, Wv):
    x = np.asarray(x)
    B = x.shape[0]
    assert B == N_CORES and x.shape[1:] == (T, C)
    x16 = np.ascontiguousarray(x.astype(np.float16))
    Wq16 = np.ascontiguousarray(np.asarray(Wq).astype(np.float16))
    Wk16 = np.ascontiguousarray(np.asarray(Wk).astype(np.float16))
    Wv16 = np.ascontiguousarray(np.asarray(Wv).astype(np.float16))

    nc = _get_nc()
    in_maps = [{"x": x16[b], "Wq": Wq16, "Wk": Wk16, "Wv": Wv16} for b in range(B)]
    res = run_bass_kernel_spmd(nc, in_maps, core_ids=list(range(N_CORES)))
    return np.stack([r["out"] for r in res.results], axis=0)


if __name__ == "__main__":
    rng = np.random.default_rng(0)
    x = rng.standard_normal((8, T, C), dtype=np.float32)
    s = C ** -0.5
    Wq = rng.standard_normal((C, H), dtype=np.float32) * s
    Wk = rng.standard_normal((C, H), dtype=np.float32) * s
    Wv = rng.standard_normal((C, H), dtype=np.float32) * s
    out = kernel(x, Wq, Wk, Wv)
    print(out.shape, out.dtype)


# revision 18
# speedup vs baseline: 181.8327x; 181.8327x over previous
"""Causal single-head attention (B=8, T=2048, C=1024, H=128) on 8 TRN2 NeuronCores.

Sharding: data-parallel over batch B — one batch element per core; weights
replicated. Inputs are cast to fp16 on the host (halves DMA, full-rate PE);
all matmuls accumulate in fp32 PSUM, softmax/normalization in fp32.

Per-core kernel:
  phase 1: x^T tiles via PE transposes; q^T,k^T = W.T @ x^T ([H,T] layout);
           v natural [T,H] via v^T + PE transposes.
  phase 2 (per 512-query block): s^T chunk = k_chunk @ q^T  -> exp (ACT,
           scale=C^-0.5; no max subtraction needed: |s/32| < ~2.5) ->
           causal mask on diagonal chunks (gpsimd affine_select) ->
           l += ones.T @ p^T and o^T += v_chunk.T @ p^T (PSUM accum) ->
           epilogue: transpose o^T -> o, scale rows by 1/l, DMA out.
"""
import numpy as np

import concourse.bass as bass
import concourse.mybir as mybir
import concourse.tile as tile
from concourse import bacc
from concourse.bass_utils import run_bass_kernel_spmd
from concourse.masks import make_identity

P = 128
T = 2048
C = 1024
H = 128
CO = C // P          # 8 contraction chunks
TB = 512             # T block for phase 1
NTB = T // TB        # 4
QB = 512             # query block for phase 2
NQB = T // QB        # 4
NKC = T // P         # 16 key chunks
F32 = mybir.dt.float32
F16 = mybir.dt.float16
SCALE = C ** -0.5    # 1/32, matches reference (scales by n_embed, not head_size)

N_CORES = 8


def _copy(nc, idx, out, in_):
    """Alternate psum->sbuf copies between DVE and ACT to halve copy pressure."""
    if idx % 2 == 0:
        nc.vector.tensor_copy(out, in_)
    else:
        nc.scalar.activation(out, in_, mybir.ActivationFunctionType.Copy)


def build_nc(s_bufs=4, misc_bufs=2, stage_bufs=3, ptile_bufs=6, xload_bufs=16):
    nc = bacc.Bacc("TRN2", target_bir_lowering=False, debug=False,
                   enable_asserts=False, num_devices=N_CORES)
    x = nc.dram_tensor("x", [T, C], F16, kind="ExternalInput")
    wq = nc.dram_tensor("Wq", [C, H], F16, kind="ExternalInput")
    wk = nc.dram_tensor("Wk", [C, H], F16, kind="ExternalInput")
    wv = nc.dram_tensor("Wv", [C, H], F16, kind="ExternalInput")
    out = nc.dram_tensor("out", [T, H], F32, kind="ExternalOutput")

    x4 = x.rearrange("(r p) (o c) -> p r o c", p=P, c=P)    # [128, 16, 8, 128]
    out3 = out.rearrange("(n p) h -> p n h", p=P)           # [128, 16, 128]

    with tile.TileContext(nc) as tc:
        with (
            tc.tile_pool(name="const", bufs=1) as const,
            tc.tile_pool(name="persist", bufs=1) as persist,
            tc.tile_pool(name="xload", bufs=xload_bufs) as xload,
            tc.tile_pool(name="stage", bufs=stage_bufs) as stage,
            tc.tile_pool(name="ptile", bufs=ptile_bufs) as ptile,
            tc.tile_pool(name="epi", bufs=2) as epi,
            tc.tile_pool(name="ps_acc", bufs=1, space="PSUM") as ps_acc,
            tc.tile_pool(name="ps_s", bufs=s_bufs, space="PSUM") as ps_s,
            tc.tile_pool(name="ps_misc", bufs=misc_bufs, space="PSUM") as ps_misc,
        ):
            # ---- constants ----
            ident = const.tile([P, P], F32)
            make_identity(nc, ident)
            identh = const.tile([P, P], F16)
            nc.vector.tensor_copy(identh[:], ident[:])
            ones_f = const.tile([P, 1], F32)
            nc.gpsimd.memset(ones_f[:], 1.0)
            ones_h = const.tile([P, 1], F16)
            nc.vector.tensor_copy(ones_h[:], ones_f[:])

            # ---- persistent activations ----
            q_T = persist.tile([P, T], F16)          # [H, T]
            k_T = persist.tile([P, T], F16)          # [H, T]
            v_nat = persist.tile([P, NKC, H], F16)   # [t%128, kc, H]

            # ---- x loads for tb0 first (startup latency), then weights ----
            x_blks = {}

            def load_xb(i):
                halves = []
                for half in range(2):
                    xh = xload.tile([P, 4, P], F16, name="xh")  # [t, o_half, c]
                    nc.sync.dma_start(xh[:], x4[:, i, half * 4:(half + 1) * 4, :])
                    halves.append(xh)
                x_blks[i] = halves

            for r in range(4):
                load_xb(r)

            w_tiles = []
            for nm, wd in (("wqt", wq), ("wkt", wk), ("wvt", wv)):
                wt = const.tile([P, CO, H], F16, name=nm)
                nc.sync.dma_start(wt[:], wd.rearrange("(o p) h -> p o h", p=P))
                w_tiles.append(wt)
            wq_t, wk_t, wv_t = w_tiles

            # ================= phase 1: projections =================
            cpy = 0
            for tb in range(NTB):
                xT = stage.tile([P, CO, TB], F16, name="xT")  # [c_in_chunk, o, t]
                for r in range(4):
                    if tb * 4 + r not in x_blks:
                        load_xb(tb * 4 + r)
                # per r-block: transpose its 8 chunks (2 psum tiles), copy out
                for r in range(4):
                    for half in range(2):
                        xh = x_blks[tb * 4 + r][half]
                        ps_x = ps_misc.tile([P, TB], F16, name="ps_x", tag="mps")
                        for cl in range(4):
                            nc.tensor.transpose(
                                ps_x[:, cl * P:(cl + 1) * P], xh[:, cl, :], identh[:])
                        _copy(nc, cpy, xT[:, half * 4:(half + 1) * 4, r * P:(r + 1) * P],
                              ps_x[:].rearrange("p (c t) -> p c t", t=P))
                        cpy += 1

                tsl = slice(tb * TB, (tb + 1) * TB)
                for wt, dest in ((wq_t, q_T), (wk_t, k_T)):
                    ps_p = ps_misc.tile([P, TB], F32, name="ps_p", tag="mps")
                    for c in range(CO):
                        nc.tensor.matmul(ps_p[:], wt[:, c, :], xT[:, c, :],
                                         start=(c == 0), stop=(c == CO - 1))
                    _copy(nc, cpy, dest[:, tsl], ps_p[:])
                    cpy += 1

                # v directly in natural layout: v_sub = x_sub @ Wv (fp16, N=128)
                ps_v = ps_misc.tile([P, TB], F32, name="ps_v", tag="mps")
                for j in range(4):
                    for c in range(CO):
                        nc.tensor.matmul(
                            ps_v[:, j * P:(j + 1) * P],
                            xT[:, c, j * P:(j + 1) * P], wv_t[:, c, :],
                            start=(c == 0), stop=(c == CO - 1))
                _copy(nc, cpy, v_nat[:, tb * 4:(tb + 1) * 4, :],
                      ps_v[:].rearrange("p (j h) -> p j h", h=H))
                cpy += 1

            # ================= phase 2: attention =================
            for b in range(NQB):
                nkc = 4 * (b + 1)
                o_ps = ps_acc.tile([P, QB], F32, name="o_ps")
                l_ps = ps_acc.tile([1, QB], F32, name="l_ps")
                for kc in range(nkc):
                    d = kc - 4 * b
                    off = max(d, 0) * P      # diagonal chunks: only queries >= key chunk start
                    w = QB - off
                    s_ps = ps_s.tile([P, QB], F32, name="s_ps")
                    nc.tensor.matmul(s_ps[:, :w], k_T[:, kc * P:(kc + 1) * P],
                                     q_T[:, b * QB + off:(b + 1) * QB],
                                     start=True, stop=True)
                    pT = ptile.tile([P, QB], F16, name="pT")
                    nc.scalar.activation(pT[:, off:], s_ps[:, :w],
                                         mybir.ActivationFunctionType.Exp, scale=SCALE)
                    if d >= 0:  # diagonal chunk: zero where key > query
                        nc.gpsimd.affine_select(
                            out=pT[:, off:], in_=pT[:, off:],
                            compare_op=mybir.AluOpType.is_ge,
                            fill=0.0, base=0,
                            pattern=[[1, w]], channel_multiplier=-1)
                    nc.tensor.matmul(l_ps[:, off:], ones_h[:], pT[:, off:],
                                     start=(kc == 0), stop=(kc == nkc - 1))
                    nc.tensor.matmul(o_ps[:, off:], v_nat[:, kc, :], pT[:, off:],
                                     start=(kc == 0), stop=(kc == nkc - 1))

                # epilogue: l -> linv [128, 4]; o^T -> o natural; scale; DMA out
                l_sb = epi.tile([1, QB], F32, name="l_sb")
                nc.vector.tensor_copy(l_sb[:], l_ps[:])
                ps_l = ps_misc.tile([P, 4], F32, name="ps_l", tag="mps")
                for j in range(4):
                    nc.tensor.transpose(ps_l[:, j:j + 1], l_sb[:, j * P:(j + 1) * P],
                                        ident[:1, :1])
                l_nat = epi.tile([P, 4], F32, name="l_nat")
                nc.vector.tensor_copy(l_nat[:], ps_l[:])
                linv = epi.tile([P, 4], F32, name="linv")
                nc.vector.reciprocal(linv[:], l_nat[:])

                oT_sb = epi.tile([P, QB], F32, name="oT_sb")
                o_nat = epi.tile([P, 4, H], F32, name="o_nat")
                for hf in range(2):
                    hsl = slice(hf * (QB // 2), (hf + 1) * (QB // 2))
                    nc.vector.tensor_copy(oT_sb[:, hsl], o_ps[:, hsl])
                    ps_on = ps_misc.tile([P, QB // 2], F32, name="ps_on", tag="mps")
                    for jj in range(2):
                        j = hf * 2 + jj
                        nc.tensor.transpose(
                            ps_on[:, jj * P:(jj + 1) * P], oT_sb[:, j * P:(j + 1) * P],
                            ident[:])
                    nc.vector.tensor_tensor(
                        o_nat[:, hf * 2:(hf + 1) * 2, :],
                        ps_on[:].rearrange("p (j h) -> p j h", h=H),
                        linv[:, hf * 2:(hf + 1) * 2, None].to_broadcast([P, 2, H]),
                        mybir.AluOpType.mult)
                    nc.sync.dma_start(out3[:, b * 4 + hf * 2:b * 4 + (hf + 1) * 2, :],
                                      o_nat[:, hf * 2:(hf + 1) * 2, :])

    nc.compile()
    return nc


_NC = None


def _get_nc():
    global _NC
    if _NC is None:
        _NC = build_nc()
    return _NC


def kernel(x, Wq, Wk, Wv):
    x = np.asarray(x)
    B = x.shape[0]
    assert B == N_CORES and x.shape[1:] == (T, C)
    x16 = np.ascontiguousarray(x.astype(np.float16))
    Wq16 = np.ascontiguousarray(np.asarray(Wq).astype(np.float16))
    Wk16 = np.ascontiguousarray(np.asarray(Wk).astype(np.float16))
    Wv16 = np.ascontiguousarray(np.asarray(Wv).astype(np.float16))

    nc = _get_nc()
    in_maps = [{"x": x16[b], "Wq": Wq16, "Wk": Wk16, "Wv": Wv16} for b in range(B)]
    res = run_bass_kernel_spmd(nc, in_maps, core_ids=list(range(N_CORES)))
    return np.stack([r["out"] for r in res.results], axis=0)


if __name__ == "__main__":
    rng = np.random.default_rng(0)
    x = rng.standard_normal((8, T, C), dtype=np.float32)
    s = C ** -0.5
    Wq = rng.standard_normal((C, H), dtype=np.float32) * s
    Wk = rng.standard_normal((C, H), dtype=np.float32) * s
    Wv = rng.standard_normal((C, H), dtype=np.float32) * s
    out = kernel(x, Wq, Wk, Wv)
    print(out.shape, out.dtype)


# revision 25
# speedup vs baseline: 189.2527x; 1.0408x over previous
"""Causal single-head attention (B=8, T=2048, C=1024, H=128) on 8 TRN2 NeuronCores.

Sharding: data-parallel over batch B — one batch element per core; weights
replicated. Inputs are cast to fp16 on the host (halves DMA, full-rate PE);
all matmuls accumulate in fp32 PSUM, softmax/normalization in fp32.

Per-core kernel:
  phase 1: x^T tiles via PE transposes; q^T,k^T = W.T @ x^T ([H,T] layout);
           v natural [T,H] via v^T + PE transposes.
  phase 2 (per 512-query block): s^T chunk = k_chunk @ q^T  -> exp (ACT,
           scale=C^-0.5; no max subtraction needed: |s/32| < ~2.5) ->
           causal mask on diagonal chunks (gpsimd affine_select) ->
           l += ones.T @ p^T and o^T += v_chunk.T @ p^T (PSUM accum) ->
           epilogue: transpose o^T -> o, scale rows by 1/l, DMA out.
"""
import numpy as np

import concourse.bass as bass
import concourse.mybir as mybir
import concourse.tile as tile
from concourse import bacc
from concourse.bass_utils import run_bass_kernel_spmd
from concourse.masks import make_identity

P = 128
T = 2048
C = 1024
H = 128
CO = C // P          # 8 contraction chunks
TB = 512             # T block for phase 1
NTB = T // TB        # 4
QB = 512             # query block for phase 2
NQB = T // QB        # 4
NKC = T // P         # 16 key chunks
F32 = mybir.dt.float32
F16 = mybir.dt.float16
SCALE = C ** -0.5    # 1/32, matches reference (scales by n_embed, not head_size)

N_CORES = 8


def _copy(nc, idx, out, in_):
    """Alternate psum->sbuf copies between DVE and ACT to halve copy pressure."""
    if idx % 2 == 0:
        nc.vector.tensor_copy(out, in_)
    else:
        nc.scalar.activation(out, in_, mybir.ActivationFunctionType.Copy)


def build_nc(s_bufs=3, misc_bufs=3, stage_bufs=4, ptile_bufs=5, xload_bufs=8):
    nc = bacc.Bacc("TRN2", target_bir_lowering=False, debug=False,
                   enable_asserts=False, num_devices=N_CORES)
    x = nc.dram_tensor("x", [T, C], F16, kind="ExternalInput")
    wq = nc.dram_tensor("Wq", [C, H], F16, kind="ExternalInput")
    wk = nc.dram_tensor("Wk", [C, H], F16, kind="ExternalInput")
    wv = nc.dram_tensor("Wv", [C, H], F16, kind="ExternalInput")
    out = nc.dram_tensor("out", [T, H], F32, kind="ExternalOutput")

    x4 = x.rearrange("(r p) (o c) -> p r o c", p=P, c=P)    # [128, 16, 8, 128]
    out3 = out.rearrange("(n p) h -> p n h", p=P)           # [128, 16, 128]

    with tile.TileContext(nc) as tc:
        with (
            tc.tile_pool(name="const", bufs=1) as const,
            tc.tile_pool(name="persist", bufs=1) as persist,
            tc.tile_pool(name="xload", bufs=8) as xload,
            tc.tile_pool(name="stage", bufs=stage_bufs) as stage,
            tc.tile_pool(name="ptile", bufs=ptile_bufs) as ptile,
            tc.tile_pool(name="epi", bufs=2) as epi,
            tc.tile_pool(name="ps_acc", bufs=1, space="PSUM") as ps_acc,
            tc.tile_pool(name="ps_s", bufs=s_bufs, space="PSUM") as ps_s,
            tc.tile_pool(name="ps_misc", bufs=misc_bufs, space="PSUM") as ps_misc,
        ):
            # ---- constants ----
            ident = const.tile([P, P], F32)
            make_identity(nc, ident)
            identh = const.tile([P, P], F16)
            nc.vector.tensor_copy(identh[:], ident[:])
            ones_f = const.tile([P, 1], F32)
            nc.gpsimd.memset(ones_f[:], 1.0)
            ones_h = const.tile([P, 1], F16)
            nc.vector.tensor_copy(ones_h[:], ones_f[:])

            # ---- persistent activations ----
            q_T = persist.tile([P, T], F16)          # [H, T]
            k_T = persist.tile([P, T], F16)          # [H, T]
            v_nat = persist.tile([P, NKC, H], F16)   # [t%128, kc, H]

            # ---- x/W loads: tb0 halves first, W halves interleaved so the
            # ---- first projections can start as early as possible ----
            x_blks = {}

            def load_xb(i):
                xb = xload.tile([P, CO, P], F16, name="xb")  # [t, o, c]
                nc.sync.dma_start(xb[:], x4[:, i])
                x_blks[i] = xb

            w_tiles = []
            w_srcs = {}
            for nm, wd in (("wqt", wq), ("wkt", wk), ("wvt", wv)):
                wt = const.tile([P, CO, H], F16, name=nm)
                w_tiles.append(wt)
                w_srcs[nm] = (wt, wd)
            wq_t, wk_t, wv_t = w_tiles

            for r in range(4):
                load_xb(r)
            for half in range(2):
                for nm, (wt, wd) in w_srcs.items():
                    nc.sync.dma_start(
                        wt[:, half * 4:(half + 1) * 4, :],
                        wd.rearrange("(o p) h -> p o h", p=P)[:, half * 4:(half + 1) * 4, :])

            # ================= phase 1: projections =================
            cpy = 0
            for tb in range(NTB):
                xT = stage.tile([P, CO, TB], F16, name="xT")  # [c_in_chunk, o, t]
                for r in range(4):
                    if tb * 4 + r not in x_blks:
                        load_xb(tb * 4 + r)
                # c-major: per chunk, transpose all 4 r-tiles -> contiguous xT[:, c, :]
                for c in range(CO):
                    ps_x = ps_misc.tile([P, TB], F16, name="ps_x", tag="mps")
                    for r in range(4):
                        nc.tensor.transpose(
                            ps_x[:, r * P:(r + 1) * P],
                            x_blks[tb * 4 + r][:, c, :], identh[:])
                    _copy(nc, cpy, xT[:, c, :], ps_x[:])
                    cpy += 1

                tsl = slice(tb * TB, (tb + 1) * TB)
                for wt, dest in ((wq_t, q_T), (wk_t, k_T)):
                    ps_p = ps_misc.tile([P, TB], F32, name="ps_p", tag="mps")
                    for c in range(CO):
                        nc.tensor.matmul(ps_p[:], wt[:, c, :], xT[:, c, :],
                                         start=(c == 0), stop=(c == CO - 1))
                    _copy(nc, cpy, dest[:, tsl], ps_p[:])
                    cpy += 1

                # v directly in natural layout: v_sub = x_sub @ Wv (fp16, N=128)
                ps_v = ps_misc.tile([P, TB], F32, name="ps_v", tag="mps")
                for j in range(4):
                    for c in range(CO):
                        nc.tensor.matmul(
                            ps_v[:, j * P:(j + 1) * P],
                            xT[:, c, j * P:(j + 1) * P], wv_t[:, c, :],
                            start=(c == 0), stop=(c == CO - 1))
                _copy(nc, cpy, v_nat[:, tb * 4:(tb + 1) * 4, :],
                      ps_v[:].rearrange("p (j h) -> p j h", h=H))
                cpy += 1

            # ================= phase 2: attention =================
            for b in range(NQB):
                nkc = 4 * (b + 1)
                o_ps = ps_acc.tile([P, QB], F32, name="o_ps")
                l_ps = ps_acc.tile([1, QB], F32, name="l_ps")
                kc_order = list(range(4 * b, nkc)) + list(range(0, 4 * b))
                for kc in kc_order:
                    d = kc - 4 * b
                    off = max(d, 0) * P      # diagonal chunks: only queries >= key chunk start
                    w = QB - off
                    s_ps = ps_s.tile([P, QB], F32, name="s_ps")
                    nc.tensor.matmul(s_ps[:, :w], k_T[:, kc * P:(kc + 1) * P],
                                     q_T[:, b * QB + off:(b + 1) * QB],
                                     start=True, stop=True)
                    pT = ptile.tile([P, QB], F16, name="pT")
                    nc.scalar.activation(pT[:, off:], s_ps[:, :w],
                                         mybir.ActivationFunctionType.Exp, scale=SCALE)
                    if d >= 0:  # diagonal chunk: zero where key > query
                        nc.gpsimd.affine_select(
                            out=pT[:, off:], in_=pT[:, off:],
                            compare_op=mybir.AluOpType.is_ge,
                            fill=0.0, base=0,
                            pattern=[[1, w]], channel_multiplier=-1)
                    first = kc == kc_order[0]
                    last = kc == kc_order[-1]
                    nc.tensor.matmul(l_ps[:, off:], ones_h[:], pT[:, off:],
                                     start=first, stop=last)
                    nc.tensor.matmul(o_ps[:, off:], v_nat[:, kc, :], pT[:, off:],
                                     start=first, stop=last)

                # epilogue: l -> linv [128, 4]; o^T -> o natural; scale; DMA out
                l_sb = epi.tile([1, QB], F32, name="l_sb")
                if b == NQB - 1:
                    nc.scalar.activation(l_sb[:], l_ps[:],
                                         mybir.ActivationFunctionType.Copy)
                else:
                    nc.vector.tensor_copy(l_sb[:], l_ps[:])
                ps_l = ps_misc.tile([P, 4], F32, name="ps_l", tag="mps")
                for j in range(4):
                    nc.tensor.transpose(ps_l[:, j:j + 1], l_sb[:, j * P:(j + 1) * P],
                                        ident[:1, :1])
                l_nat = epi.tile([P, 4], F32, name="l_nat")
                nc.vector.tensor_copy(l_nat[:], ps_l[:])
                linv = epi.tile([P, 4], F32, name="linv")
                nc.vector.reciprocal(linv[:], l_nat[:])

                oT_sb = epi.tile([P, QB], F32, name="oT_sb")
                o_nat = epi.tile([P, 4, H], F32, name="o_nat")
                for hf in range(2):
                    hsl = slice(hf * (QB // 2), (hf + 1) * (QB // 2))
                    if b == NQB - 1 and hf == 1:
                        nc.scalar.activation(oT_sb[:, hsl], o_ps[:, hsl],
                                             mybir.ActivationFunctionType.Copy)
                    else:
                        nc.vector.tensor_copy(oT_sb[:, hsl], o_ps[:, hsl])
                    ps_on = ps_misc.tile([P, QB // 2], F32, name="ps_on", tag="mps")
                    for jj in range(2):
                        j = hf * 2 + jj
                        nc.tensor.transpose(
                            ps_on[:, jj * P:(jj + 1) * P], oT_sb[:, j * P:(j + 1) * P],
                            ident[:])
                    nc.vector.tensor_tensor(
                        o_nat[:, hf * 2:(hf + 1) * 2, :],
                        ps_on[:].rearrange("p (j h) -> p j h", h=H),
                        linv[:, hf * 2:(hf + 1) * 2, None].to_broadcast([P, 2, H]),
                        mybir.AluOpType.mult)
                    nc.sync.dma_start(out3[:, b * 4 + hf * 2:b * 4 + (hf + 1) * 2, :],
                                      o_nat[:, hf * 2:(hf + 1) * 2, :])

    nc.compile()
    return nc


_NC = None


def _get_nc():
    global _NC
    if _NC is None:
        _NC = build_nc()
    return _NC


def kernel(x, Wq, Wk, Wv):
    x = np.asarray(x)
    B = x.shape[0]
    assert B == N_CORES and x.shape[1:] == (T, C)
    x16 = np.ascontiguousarray(x.astype(np.float16))
    Wq16 = np.ascontiguousarray(np.asarray(Wq).astype(np.float16))
    Wk16 = np.ascontiguousarray(np.asarray(Wk).astype(np.float16))
    Wv16 = np.ascontiguousarray(np.asarray(Wv).astype(np.float16))

    nc = _get_nc()
    in_maps = [{"x": x16[b], "Wq": Wq16, "Wk": Wk16, "Wv": Wv16} for b in range(B)]
    res = run_bass_kernel_spmd(nc, in_maps, core_ids=list(range(N_CORES)))
    return np.stack([r["out"] for r in res.results], axis=0)


if __name__ == "__main__":
    rng = np.random.default_rng(0)
    x = rng.standard_normal((8, T, C), dtype=np.float32)
    s = C ** -0.5
    Wq = rng.standard_normal((C, H), dtype=np.float32) * s
    Wk = rng.standard_normal((C, H), dtype=np.float32) * s
    Wv = rng.standard_normal((C, H), dtype=np.float32) * s
    out = kernel(x, Wq, Wk, Wv)
    print(out.shape, out.dtype)


# revision 27
# speedup vs baseline: 192.0646x; 1.0149x over previous
"""Causal single-head attention (B=8, T=2048, C=1024, H=128) on 8 TRN2 NeuronCores.

Sharding: data-parallel over batch B — one batch element per core; weights
replicated. Inputs are cast to fp16 on the host (halves DMA, full-rate PE);
all matmuls accumulate in fp32 PSUM, softmax/normalization in fp32.

Per-core kernel:
  phase 1: x^T tiles via PE transposes; q^T,k^T = W.T @ x^T ([H,T] layout);
           v natural [T,H] via v^T + PE transposes.
  phase 2 (per 512-query block): s^T chunk = k_chunk @ q^T  -> exp (ACT,
           scale=C^-0.5; no max subtraction needed: |s/32| < ~2.5) ->
           causal mask on diagonal chunks (gpsimd affine_select) ->
           l += ones.T @ p^T and o^T += v_chunk.T @ p^T (PSUM accum) ->
           epilogue: transpose o^T -> o, scale rows by 1/l, DMA out.
"""
import numpy as np

import concourse.bass as bass
import concourse.mybir as mybir
import concourse.tile as tile
from concourse import bacc
from concourse.bass_utils import run_bass_kernel_spmd
from concourse.masks import make_identity

P = 128
T = 2048
C = 1024
H = 128
CO = C // P          # 8 contraction chunks
TB = 512             # T block for phase 1
NTB = T // TB        # 4
QB = 512             # query block for phase 2
NQB = T // QB        # 4
NKC = T // P         # 16 key chunks
F32 = mybir.dt.float32
F16 = mybir.dt.float16
SCALE = C ** -0.5    # 1/32, matches reference (scales by n_embed, not head_size)

N_CORES = 8


def _copy(nc, idx, out, in_):
    """Alternate psum->sbuf copies between DVE and ACT to halve copy pressure."""
    if idx % 2 == 0:
        nc.vector.tensor_copy(out, in_)
    else:
        nc.scalar.activation(out, in_, mybir.ActivationFunctionType.Copy)


def build_nc(s_bufs=3, misc_bufs=3, stage_bufs=4, ptile_bufs=4, xload_bufs=8):
    nc = bacc.Bacc("TRN2", target_bir_lowering=False, debug=False,
                   enable_asserts=False, num_devices=N_CORES)
    x = nc.dram_tensor("x", [T, C], F16, kind="ExternalInput")
    wq = nc.dram_tensor("Wq", [C, H], F16, kind="ExternalInput")
    wk = nc.dram_tensor("Wk", [C, H], F16, kind="ExternalInput")
    wv = nc.dram_tensor("Wv", [C, H], F16, kind="ExternalInput")
    out = nc.dram_tensor("out", [T, H], F32, kind="ExternalOutput")

    x4 = x.rearrange("(r p) (o c) -> p r o c", p=P, c=P)    # [128, 16, 8, 128]
    out3 = out.rearrange("(n p) h -> p n h", p=P)           # [128, 16, 128]

    with tile.TileContext(nc) as tc:
        with (
            tc.tile_pool(name="const", bufs=1) as const,
            tc.tile_pool(name="persist", bufs=1) as persist,
            tc.tile_pool(name="xload", bufs=8) as xload,
            tc.tile_pool(name="stage", bufs=stage_bufs) as stage,
            tc.tile_pool(name="ptile", bufs=ptile_bufs) as ptile,
            tc.tile_pool(name="epi", bufs=3) as epi,
            tc.tile_pool(name="ps_acc", bufs=1, space="PSUM") as ps_acc,
            tc.tile_pool(name="ps_s", bufs=s_bufs, space="PSUM") as ps_s,
            tc.tile_pool(name="ps_misc", bufs=misc_bufs, space="PSUM") as ps_misc,
        ):
            # ---- constants ----
            ident = const.tile([P, P], F32)
            make_identity(nc, ident)
            identh = const.tile([P, P], F16)
            nc.vector.tensor_copy(identh[:], ident[:])
            ones_f = const.tile([P, 1], F32)
            nc.gpsimd.memset(ones_f[:], 1.0)
            ones_h = const.tile([P, 1], F16)
            nc.vector.tensor_copy(ones_h[:], ones_f[:])

            # ---- persistent activations ----
            q_T = persist.tile([P, T], F16)          # [H, T]
            k_T = persist.tile([P, T], F16)          # [H, T]
            v_nat = persist.tile([P, NKC, H], F16)   # [t%128, kc, H]

            # ---- x/W loads: tb0 halves first, W halves interleaved so the
            # ---- first projections can start as early as possible ----
            x_blks = {}

            def load_xb(i):
                xb = xload.tile([P, CO, P], F16, name="xb")  # [t, o, c]
                nc.sync.dma_start(xb[:], x4[:, i])
                x_blks[i] = xb

            w_tiles = []
            w_srcs = {}
            for nm, wd in (("wqt", wq), ("wkt", wk), ("wvt", wv)):
                wt = const.tile([P, CO, H], F16, name=nm)
                w_tiles.append(wt)
                w_srcs[nm] = (wt, wd)
            wq_t, wk_t, wv_t = w_tiles

            for r in range(4):
                load_xb(r)
            for half in range(2):
                for nm, (wt, wd) in w_srcs.items():
                    nc.sync.dma_start(
                        wt[:, half * 4:(half + 1) * 4, :],
                        wd.rearrange("(o p) h -> p o h", p=P)[:, half * 4:(half + 1) * 4, :])

            # ================= phase 1: projections =================
            cpy = 0
            for tb in range(NTB):
                xT = stage.tile([P, CO, TB], F16, name="xT")  # [c_in_chunk, o, t]
                for r in range(4):
                    if tb * 4 + r not in x_blks:
                        load_xb(tb * 4 + r)
                # c-major: per chunk, transpose all 4 r-tiles -> contiguous xT[:, c, :]
                for c in range(CO):
                    ps_x = ps_misc.tile([P, TB], F16, name="ps_x", tag="mps")
                    for r in range(4):
                        nc.tensor.transpose(
                            ps_x[:, r * P:(r + 1) * P],
                            x_blks[tb * 4 + r][:, c, :], identh[:])
                    _copy(nc, cpy, xT[:, c, :], ps_x[:])
                    cpy += 1

                tsl = slice(tb * TB, (tb + 1) * TB)
                for wt, dest in ((wq_t, q_T), (wk_t, k_T)):
                    ps_p = ps_misc.tile([P, TB], F32, name="ps_p", tag="mps")
                    for c in range(CO):
                        nc.tensor.matmul(ps_p[:], wt[:, c, :], xT[:, c, :],
                                         start=(c == 0), stop=(c == CO - 1))
                    _copy(nc, cpy, dest[:, tsl], ps_p[:])
                    cpy += 1

                # v directly in natural layout: v_sub = x_sub @ Wv (fp16, N=128)
                ps_v = ps_misc.tile([P, TB], F32, name="ps_v", tag="mps")
                for j in range(4):
                    for c in range(CO):
                        nc.tensor.matmul(
                            ps_v[:, j * P:(j + 1) * P],
                            xT[:, c, j * P:(j + 1) * P], wv_t[:, c, :],
                            start=(c == 0), stop=(c == CO - 1))
                _copy(nc, cpy, v_nat[:, tb * 4:(tb + 1) * 4, :],
                      ps_v[:].rearrange("p (j h) -> p j h", h=H))
                cpy += 1

            # ================= phase 2: attention =================
            for b in range(NQB):
                nkc = 4 * (b + 1)
                o_ps = ps_acc.tile([P, QB], F32, name="o_ps")
                l_ps = ps_acc.tile([1, QB], F32, name="l_ps")
                kc_order = list(range(4 * b, nkc)) + list(range(0, 4 * b))
                for kc in kc_order:
                    d = kc - 4 * b
                    off = max(d, 0) * P      # diagonal chunks: only queries >= key chunk start
                    w = QB - off
                    s_ps = ps_s.tile([P, QB], F32, name="s_ps")
                    nc.tensor.matmul(s_ps[:, :w], k_T[:, kc * P:(kc + 1) * P],
                                     q_T[:, b * QB + off:(b + 1) * QB],
                                     start=True, stop=True)
                    pT = ptile.tile([P, QB], F16, name="pT")
                    nc.scalar.activation(pT[:, off:], s_ps[:, :w],
                                         mybir.ActivationFunctionType.Exp, scale=SCALE)
                    if d >= 0:  # diagonal chunk: zero where key > query
                        nc.gpsimd.affine_select(
                            out=pT[:, off:], in_=pT[:, off:],
                            compare_op=mybir.AluOpType.is_ge,
                            fill=0.0, base=0,
                            pattern=[[1, w]], channel_multiplier=-1)
                    first = kc == kc_order[0]
                    last = kc == kc_order[-1]
                    nc.tensor.matmul(o_ps[:, off:], v_nat[:, kc, :], pT[:, off:],
                                     start=first, stop=last)
                    nc.tensor.matmul(l_ps[:, off:], ones_h[:], pT[:, off:],
                                     start=first, stop=last)

                # epilogue: l -> linv [128, 4]; o^T -> o natural; scale; DMA out
                l_sb = epi.tile([1, QB], F32, name="l_sb")
                if b == NQB - 1:
                    nc.scalar.activation(l_sb[:], l_ps[:],
                                         mybir.ActivationFunctionType.Copy)
                else:
                    nc.vector.tensor_copy(l_sb[:], l_ps[:])
                ps_l = ps_misc.tile([P, 4], F32, name="ps_l", tag="mps")
                for j in range(4):
                    nc.tensor.transpose(ps_l[:, j:j + 1], l_sb[:, j * P:(j + 1) * P],
                                        ident[:1, :1])
                l_nat = epi.tile([P, 4], F32, name="l_nat")
                nc.vector.tensor_copy(l_nat[:], ps_l[:])
                linv = epi.tile([P, 4], F32, name="linv")
                nc.vector.reciprocal(linv[:], l_nat[:])

                oT_sb = epi.tile([P, QB], F32, name="oT_sb")
                o_nat = epi.tile([P, 4, H], F32, name="o_nat")
                for hf in range(2):
                    hsl = slice(hf * (QB // 2), (hf + 1) * (QB // 2))
                    if b == NQB - 1 and hf == 1:
                        nc.scalar.activation(oT_sb[:, hsl], o_ps[:, hsl],
                                             mybir.ActivationFunctionType.Copy)
                    else:
                        nc.vector.tensor_copy(oT_sb[:, hsl], o_ps[:, hsl])
                    ps_on = ps_misc.tile([P, QB // 2], F32, name="ps_on", tag="mps")
                    for jj in range(2):
                        j = hf * 2 + jj
                        nc.tensor.transpose(
                            ps_on[:, jj * P:(jj + 1) * P], oT_sb[:, j * P:(j + 1) * P],
                            ident[:])
                    nc.vector.tensor_tensor(
                        o_nat[:, hf * 2:(hf + 1) * 2, :],
                        ps_on[:].rearrange("p (j h) -> p j h", h=H),
                        linv[:, hf * 2:(hf + 1) * 2, None].to_broadcast([P, 2, H]),
                        mybir.AluOpType.mult)
                    nc.sync.dma_start(out3[:, b * 4 + hf * 2:b * 4 + (hf + 1) * 2, :],
                                      o_nat[:, hf * 2:(hf + 1) * 2, :])

    nc.compile()
    return nc


_NC = None


def _get_nc():
    global _NC
    if _NC is None:
        _NC = build_nc()
    return _NC


def kernel(x, Wq, Wk, Wv):
    x = np.asarray(x)
    B = x.shape[0]
    assert B == N_CORES and x.shape[1:] == (T, C)
    x16 = np.ascontiguousarray(x.astype(np.float16))
    Wq16 = np.ascontiguousarray(np.asarray(Wq).astype(np.float16))
    Wk16 = np.ascontiguousarray(np.asarray(Wk).astype(np.float16))
    Wv16 = np.ascontiguousarray(np.asarray(Wv).astype(np.float16))

    nc = _get_nc()
    in_maps = [{"x": x16[b], "Wq": Wq16, "Wk": Wk16, "Wv": Wv16} for b in range(B)]
    res = run_bass_kernel_spmd(nc, in_maps, core_ids=list(range(N_CORES)))
    return np.stack([r["out"] for r in res.results], axis=0)


if __name__ == "__main__":
    rng = np.random.default_rng(0)
    x = rng.standard_normal((8, T, C), dtype=np.float32)
    s = C ** -0.5
    Wq = rng.standard_normal((C, H), dtype=np.float32) * s
    Wk = rng.standard_normal((C, H), dtype=np.float32) * s
    Wv = rng.standard_normal((C, H), dtype=np.float32) * s
    out = kernel(x, Wq, Wk, Wv)
    print(out.shape, out.dtype)


# revision 33
# speedup vs baseline: 193.0718x; 1.0052x over previous
"""Causal single-head attention (B=8, T=2048, C=1024, H=128) on 8 TRN2 NeuronCores.

Sharding: data-parallel over batch B — one batch element per core; weights
replicated. Inputs are cast to fp16 on the host (halves DMA, full-rate PE);
all matmuls accumulate in fp32 PSUM, softmax/normalization in fp32.

Per-core kernel:
  phase 1: x^T tiles via PE transposes; q^T,k^T = W.T @ x^T ([H,T] layout);
           v natural [T,H] via v^T + PE transposes.
  phase 2 (per 512-query block): s^T chunk = k_chunk @ q^T  -> exp (ACT,
           scale=C^-0.5; no max subtraction needed: |s/32| < ~2.5) ->
           causal mask on diagonal chunks (gpsimd affine_select) ->
           l += ones.T @ p^T and o^T += v_chunk.T @ p^T (PSUM accum) ->
           epilogue: transpose o^T -> o, scale rows by 1/l, DMA out.
"""
import numpy as np

import concourse.bass as bass
import concourse.mybir as mybir
import concourse.tile as tile
from concourse import bacc
from concourse.bass_utils import run_bass_kernel_spmd
from concourse.masks import make_identity

P = 128
T = 2048
C = 1024
H = 128
CO = C // P          # 8 contraction chunks
TB = 512             # T block for phase 1
NTB = T // TB        # 4
QB = 512             # query block for phase 2
NQB = T // QB        # 4
NKC = T // P         # 16 key chunks
F32 = mybir.dt.float32
F16 = mybir.dt.float16
SCALE = C ** -0.5    # 1/32, matches reference (scales by n_embed, not head_size)

N_CORES = 8


def _copy(nc, idx, out, in_):
    """Alternate psum->sbuf copies between DVE and ACT to halve copy pressure."""
    if idx % 2 == 0:
        nc.vector.tensor_copy(out, in_)
    else:
        nc.scalar.activation(out, in_, mybir.ActivationFunctionType.Copy)


def build_nc(s_bufs=3, misc_bufs=3, stage_bufs=4, ptile_bufs=4, xload_bufs=8):
    nc = bacc.Bacc("TRN2", target_bir_lowering=False, debug=False,
                   enable_asserts=False, num_devices=N_CORES)
    x = nc.dram_tensor("x", [T, C], F16, kind="ExternalInput")
    wq = nc.dram_tensor("Wq", [C, H], F16, kind="ExternalInput")
    wk = nc.dram_tensor("Wk", [C, H], F16, kind="ExternalInput")
    wv = nc.dram_tensor("Wv", [C, H], F16, kind="ExternalInput")
    out = nc.dram_tensor("out", [T, H], F32, kind="ExternalOutput")

    x4 = x.rearrange("(r p) (o c) -> p r o c", p=P, c=P)    # [128, 16, 8, 128]
    out3 = out.rearrange("(n p) h -> p n h", p=P)           # [128, 16, 128]

    with tile.TileContext(nc) as tc:
        with (
            tc.tile_pool(name="const", bufs=1) as const,
            tc.tile_pool(name="persist", bufs=1) as persist,
            tc.tile_pool(name="xload", bufs=8) as xload,
            tc.tile_pool(name="stage", bufs=stage_bufs) as stage,
            tc.tile_pool(name="ptile", bufs=ptile_bufs) as ptile,
            tc.tile_pool(name="epi", bufs=3) as epi,
            tc.tile_pool(name="ps_acc", bufs=1, space="PSUM") as ps_acc,
            tc.tile_pool(name="ps_s", bufs=s_bufs, space="PSUM") as ps_s,
            tc.tile_pool(name="ps_misc", bufs=misc_bufs, space="PSUM") as ps_misc,
        ):
            # ---- constants ----
            ident = const.tile([P, P], F32)
            make_identity(nc, ident)
            identh = const.tile([P, P], F16)
            nc.vector.tensor_copy(identh[:], ident[:])
            ones_f = const.tile([P, 1], F32)
            nc.gpsimd.memset(ones_f[:], 1.0)
            ones_h = const.tile([P, 1], F16)
            nc.vector.tensor_copy(ones_h[:], ones_f[:])

            # ---- persistent activations ----
            q_T = persist.tile([P, T], F16)          # [H, T]
            k_T = persist.tile([P, T], F16)          # [H, T]
            v_nat = persist.tile([P, NKC, H], F16)   # [t%128, kc, H]

            # ---- x/W loads: tb0 halves first, W halves interleaved so the
            # ---- first projections can start as early as possible ----
            x_blks = {}

            def load_xb(i):
                xb = xload.tile([P, CO, P], F16, name="xb")  # [t, o, c]
                nc.sync.dma_start(xb[:], x4[:, i])
                x_blks[i] = xb

            w_tiles = []
            w_srcs = {}
            for nm, wd in (("wqt", wq), ("wkt", wk), ("wvt", wv)):
                wt = const.tile([P, CO, H], F16, name=nm)
                w_tiles.append(wt)
                w_srcs[nm] = (wt, wd)
            wq_t, wk_t, wv_t = w_tiles

            for r in range(4):
                load_xb(r)
            for half in range(2):
                for nm, (wt, wd) in w_srcs.items():
                    nc.sync.dma_start(
                        wt[:, half * 4:(half + 1) * 4, :],
                        wd.rearrange("(o p) h -> p o h", p=P)[:, half * 4:(half + 1) * 4, :])

            # ================= phase 1: projections =================
            cpy = 0
            for tb in range(NTB):
                xT = stage.tile([P, CO, TB], F16, name="xT")  # [c_in_chunk, o, t]
                for r in range(4):
                    if tb * 4 + r not in x_blks:
                        load_xb(tb * 4 + r)
                # c-major: per chunk, transpose all 4 r-tiles -> contiguous xT[:, c, :]
                for c in range(CO):
                    ps_x = ps_misc.tile([P, TB], F16, name="ps_x", tag="mps")
                    for r in range(4):
                        nc.tensor.transpose(
                            ps_x[:, r * P:(r + 1) * P],
                            x_blks[tb * 4 + r][:, c, :], identh[:])
                    _copy(nc, cpy, xT[:, c, :], ps_x[:])
                    cpy += 1

                tsl = slice(tb * TB, (tb + 1) * TB)
                for wt, dest in ((wq_t, q_T), (wk_t, k_T)):
                    ps_p = ps_misc.tile([P, TB], F32, name="ps_p", tag="mps")
                    for c in range(CO):
                        nc.tensor.matmul(ps_p[:], wt[:, c, :], xT[:, c, :],
                                         start=(c == 0), stop=(c == CO - 1))
                    _copy(nc, cpy, dest[:, tsl], ps_p[:])
                    cpy += 1

                # v directly in natural layout: v_sub = x_sub @ Wv (fp16, N=128)
                ps_v = ps_misc.tile([P, TB], F32, name="ps_v", tag="mps")
                for j in range(4):
                    for c in range(CO):
                        nc.tensor.matmul(
                            ps_v[:, j * P:(j + 1) * P],
                            xT[:, c, j * P:(j + 1) * P], wv_t[:, c, :],
                            start=(c == 0), stop=(c == CO - 1))
                _copy(nc, cpy, v_nat[:, tb * 4:(tb + 1) * 4, :],
                      ps_v[:].rearrange("p (j h) -> p j h", h=H))
                cpy += 1

            # ================= phase 2: attention =================
            for b in range(NQB):
                nkc = 4 * (b + 1)
                o_ps = ps_acc.tile([P, QB], F32, name="o_ps")
                l_ps = ps_acc.tile([1, QB], F32, name="l_ps")
                kc_order = list(range(4 * b, nkc)) + list(range(0, 4 * b))
                for kc in kc_order:
                    d = kc - 4 * b
                    off = max(d, 0) * P      # diagonal chunks: only queries >= key chunk start
                    w = QB - off
                    s_ps = ps_s.tile([P, QB], F32, name="s_ps")
                    nc.tensor.matmul(s_ps[:, :w], k_T[:, kc * P:(kc + 1) * P],
                                     q_T[:, b * QB + off:(b + 1) * QB],
                                     start=True, stop=True)
                    pT = ptile.tile([P, QB], F16, name="pT")
                    nc.scalar.activation(pT[:, off:], s_ps[:, :w],
                                         mybir.ActivationFunctionType.Exp, scale=SCALE)
                    if d >= 0:  # diagonal chunk: zero where key > query
                        nc.gpsimd.affine_select(
                            out=pT[:, off:], in_=pT[:, off:],
                            compare_op=mybir.AluOpType.is_ge,
                            fill=0.0, base=0,
                            pattern=[[1, w]], channel_multiplier=-1)
                    first = kc == kc_order[0]
                    last = kc == kc_order[-1]
                    nc.tensor.matmul(o_ps[:, off:], v_nat[:, kc, :], pT[:, off:],
                                     start=first, stop=last)
                    nc.tensor.matmul(l_ps[:, off:], ones_h[:], pT[:, off:],
                                     start=first, stop=last)

                # epilogue: l -> linv [128, 4]; o^T -> o natural; scale; DMA out
                l_sb = epi.tile([1, QB], F32, name="l_sb")
                if b == NQB - 1:
                    nc.scalar.activation(l_sb[:], l_ps[:],
                                         mybir.ActivationFunctionType.Copy)
                else:
                    nc.vector.tensor_copy(l_sb[:], l_ps[:])
                ps_l = ps_misc.tile([P, 4], F32, name="ps_l", tag="mps")
                for j in range(4):
                    nc.tensor.transpose(ps_l[:, j:j + 1], l_sb[:, j * P:(j + 1) * P],
                                        ident[:1, :1])
                l_nat = epi.tile([P, 4], F32, name="l_nat")
                nc.vector.tensor_copy(l_nat[:], ps_l[:])
                linv = epi.tile([P, 4], F32, name="linv")
                nc.vector.reciprocal(linv[:], l_nat[:])

                oT_sb = epi.tile([P, QB], F16, name="oT_sb")
                o_nat = epi.tile([P, 4, H], F32, name="o_nat")
                if b == NQB - 1:
                    # single-shot tail epilogue: one copy, one normalize, one DMA
                    nc.vector.tensor_copy(oT_sb[:, :QB // 2], o_ps[:, :QB // 2])
                    nc.scalar.activation(oT_sb[:, QB // 2:], o_ps[:, QB // 2:],
                                         mybir.ActivationFunctionType.Copy)
                    ps_onf = ps_misc.tile([P, QB], F16, name="ps_onf", tag="mps")
                    for j in range(4):
                        nc.tensor.transpose(
                            ps_onf[:, j * P:(j + 1) * P], oT_sb[:, j * P:(j + 1) * P],
                            identh[:])
                    nc.vector.tensor_tensor(
                        o_nat[:],
                        ps_onf[:].rearrange("p (j h) -> p j h", h=H),
                        linv[:, :, None].to_broadcast([P, 4, H]),
                        mybir.AluOpType.mult)
                    nc.sync.dma_start(out3[:, b * 4:(b + 1) * 4, :], o_nat[:])
                else:
                    for hf in range(2):
                        hsl = slice(hf * (QB // 2), (hf + 1) * (QB // 2))
                        nc.vector.tensor_copy(oT_sb[:, hsl], o_ps[:, hsl])
                        ps_on = ps_misc.tile([P, QB // 2], F16, name="ps_on", tag="mps")
                        for jj in range(2):
                            j = hf * 2 + jj
                            nc.tensor.transpose(
                                ps_on[:, jj * P:(jj + 1) * P], oT_sb[:, j * P:(j + 1) * P],
                                identh[:])
                        nc.vector.tensor_tensor(
                            o_nat[:, hf * 2:(hf + 1) * 2, :],
                            ps_on[:].rearrange("p (j h) -> p j h", h=H),
                            linv[:, hf * 2:(hf + 1) * 2, None].to_broadcast([P, 2, H]),
                            mybir.AluOpType.mult)
                        nc.sync.dma_start(out3[:, b * 4 + hf * 2:b * 4 + (hf + 1) * 2, :],
                                          o_nat[:, hf * 2:(hf + 1) * 2, :])

    nc.compile()
    return nc


_NC = None


def _get_nc():
    global _NC
    if _NC is None:
        _NC = build_nc()
    return _NC


def kernel(x, Wq, Wk, Wv):
    x = np.asarray(x)
    B = x.shape[0]
    assert B == N_CORES and x.shape[1:] == (T, C)
    x16 = np.ascontiguousarray(x.astype(np.float16))
    Wq16 = np.ascontiguousarray(np.asarray(Wq).astype(np.float16))
    Wk16 = np.ascontiguousarray(np.asarray(Wk).astype(np.float16))
    Wv16 = np.ascontiguousarray(np.asarray(Wv).astype(np.float16))

    nc = _get_nc()
    in_maps = [{"x": x16[b], "Wq": Wq16, "Wk": Wk16, "Wv": Wv16} for b in range(B)]
    res = run_bass_kernel_spmd(nc, in_maps, core_ids=list(range(N_CORES)))
    return np.stack([r["out"] for r in res.results], axis=0)


if __name__ == "__main__":
    rng = np.random.default_rng(0)
    x = rng.standard_normal((8, T, C), dtype=np.float32)
    s = C ** -0.5
    Wq = rng.standard_normal((C, H), dtype=np.float32) * s
    Wk = rng.standard_normal((C, H), dtype=np.float32) * s
    Wv = rng.standard_normal((C, H), dtype=np.float32) * s
    out = kernel(x, Wq, Wk, Wv)
    print(out.shape, out.dtype)
